# revision 10
# baseline (speedup 1.0000x reference)
"""DigitCaps dynamic-routing kernel for Trainium2 (8 NeuronCores, Bass/Tile).

Strategy (pure batch data-parallelism, 64 batch rows per core):
  u_hat (B,1152,10,16) is NEVER materialized. Per routing iteration:

    s[b,(o,p)]   = sum_k x2[k,b] * (c ⊙ Ws)[k,(o,p)]      (72 fp16 K-tile matmuls)
    M2[(o,p),k]  = sum_b v[b,(o,p)] * x3[b,k]             (fp8 matmuls, FWL path;
                   the 160 (o,p) rows live as 128 + 32-padded-to-128 in two
                   PSUM banks)
    agree[o,i]   = Sel^T @ (W ⊙ M2)  with (p,s)-reduction on the PE
                   (one tensor_tensor product + one fp8 DoubleRow Sel matmul
                   per 384-col chunk, accumulated over s in PSUM)

  The only cross-core op is an AllReduce of the (10,1152) agree partials per
  routing iteration (2 total), preceded by a same-sized warmup AllReduce that
  absorbs first-collective setup.

Precision: s-matmuls in fp16 for all 3 iterations; the agree path (v, x3,
products) in fp8e4m3, W-product operand in fp8e3m4 — this error only perturbs
the routing logits b_ij and stays well inside the 2e-2 budget.

Scalar-engine activation tables: exp and sqrt live in different table sets, so
each switch costs ~1.3us. Dummy activations prefetch the upcoming set while
the scalar engine is idle, keeping table loads off the serial chains.
"""
import sys

sys.path.insert(0, "/opt/trn_rl_repo")

import numpy as np
import ml_dtypes

# ---- problem constants (hardcoded per harness contract) ----
B, I, S, O, P = 512, 1152, 8, 10, 16
IS = I * S            # 9216  contraction size, k = s*I + i
OP = O * P            # 160
NCORES = 8
BL = B // NCORES      # 64 batch rows per core
KT = IS // 128        # 72 K-tiles
IC = I // 128         # 9 i-chunks per s-group
FB = 384              # free-chunk width of the agree pipeline (I = 3*FB)
NJ = IS // FB         # 24 chunks, j = s*3 + ib
NB = 3                # i-blocks for agree PSUM accumulation
F32MAX = 512          # PSUM bank width in f32 elements

GPS_EVERY = 3         # every GPS_EVERY-th product chunk goes scalar-copy+gpsimd
SEL_LAG = 5           # chunks between product issue and its Sel matmul

_CACHE = {}


def _build_module():
    import concourse.bass as bass
    import concourse.mybir as mybir
    import concourse.tile as tile
    from concourse import bacc

    f32 = mybir.dt.float32
    bf16 = mybir.dt.bfloat16
    fp16 = mybir.dt.float16
    fp8 = mybir.dt.float8e4
    fp8w = mybir.dt.float8e3
    MUL = mybir.AluOpType.mult
    ADD = mybir.AluOpType.add
    DR = mybir.MatmulPerfMode.DoubleRow
    AF = mybir.ActivationFunctionType

    nc = bacc.Bacc(
        "TRN2",
        target_bir_lowering=False,
        debug=False,
        num_devices=NCORES,
    )

    # ---- I/O ----
    x2h_d = nc.dram_tensor("x2h", [128, KT, BL], fp16, kind="ExternalInput")
    wsh_d = nc.dram_tensor("wsh", [128, KT, OP], fp16, kind="ExternalInput")
    x3f_d = nc.dram_tensor("x3f", [BL, IS], bf16, kind="ExternalInput")
    wt2_d = nc.dram_tensor("wt2", [128, 2, IS], fp8w, kind="ExternalInput")
    sel8_d = nc.dram_tensor("sel8", [128, 2, 32], fp8, kind="ExternalInput")
    b2h_d = nc.dram_tensor("b2h", [O, OP], fp16, kind="ExternalInput")
    vout_d = nc.dram_tensor("vout", [BL, OP], f32, kind="ExternalOutput")

    with tile.TileContext(nc) as tc:
        with (
            tc.tile_pool(name="const", bufs=1) as const,
            tc.tile_pool(name="rhsbig", bufs=3) as rhsp,
            tc.tile_pool(name="prod", bufs=8) as prodp,
            tc.tile_pool(name="m2c", bufs=2) as m2cp,
            tc.tile_pool(name="cexp", bufs=2) as cexpp,
            tc.tile_pool(name="ring1", bufs=1) as ring1,
            tc.tile_pool(name="ring2", bufs=2) as ring2,
            tc.tile_pool(name="psA", bufs=1, space="PSUM") as psA,
            tc.tile_pool(name="psM", bufs=2, space="PSUM") as psM,
            tc.tile_pool(name="psG", bufs=1, space="PSUM") as psG,
            tc.tile_pool(name="dram", bufs=2, space="DRAM") as dram,
        ):
            # ---------- persistent tiles ----------
            x2h = const.tile([128, KT, BL], fp16)
            wsh = const.tile([128, KT, OP], fp16)
            x3f = const.tile([BL, IS], bf16)
            wt2 = const.tile([128, 2, IS], fp8w)
            sel8 = const.tile([128, 2, 32], fp8)
            b2h = const.tile([O, OP], fp16)
            v8 = const.tile([BL, 2 * 128], bf16)  # squash out + zero pad

            # Sized warmup AllReduce input staged FIRST on the sync queue so
            # the warmup collective runs right after the entry barrier,
            # leaving the cc stream free for the real agree AllReduce.
            warm_sb = const.tile([O, I], f32)
            nc.vector.memset(warm_sb[:], 0.0)
            warm_in = dram.tile([O, I], f32, tag="warm_in")
            warm_out = dram.tile([O, I], f32, tag="warm_out")
            nc.sync.dma_start(warm_in[:], warm_sb[:])
            nc.gpsimd.collective_compute(
                "AllReduce",
                ADD,
                replica_groups=[list(range(NCORES))],
                ins=[warm_in.opt()],
                outs=[warm_out.opt()],
            )

            # ---------- load inputs (3 parallel queues) ----------
            # scalar/gpsimd: wsh alternating s-groups (the iter0 critical
            # stream); sync: x2h then agree-phase operands.
            nc.gpsimd.dma_start(sel8[:], sel8_d[:])
            nc.gpsimd.dma_start(b2h[:], b2h_d[:])
            for q0 in range(0, 3):
                ks = slice(q0 * 3, (q0 + 1) * 3)
                nc.scalar.dma_start(wsh[:, ks, :], wsh_d[:, ks, :])
            for s in range(1, S):
                ks = slice(s * IC, (s + 1) * IC)
                eng = nc.scalar if s % 2 == 0 else nc.gpsimd
                eng.dma_start(wsh[:, ks, :], wsh_d[:, ks, :])
            for s in range(S):
                ks = slice(s * IC, (s + 1) * IC)
                nc.sync.dma_start(x2h[:, ks, :], x2h_d[:, ks, :])
            nc.sync.dma_start(x3f[:], x3f_d[:])
            JCH = 3 * FB
            for c0 in range(0, IS, JCH):
                cs = slice(c0, c0 + JCH)
                nc.sync.dma_start(wt2[:, :, cs], wt2_d[:, :, cs])

            # zero pad region of v8 (persists across iterations)
            nc.vector.memset(v8[:, OP:], 0.0)

            # bias APs for activation (float biases need pre-registered consts)
            zero_b = const.tile([128, 1], f32)
            eps_b = const.tile([128, 1], f32)
            nc.vector.memset(zero_b[:], 0.0)
            nc.vector.memset(eps_b[:], 1e-8)
            # scratch for activation-table prefetch dummies
            tscr = const.tile([1, 2], f32)
            nc.vector.memset(tscr[:], 1.0)

            cexp_prev = None  # fp16 (128, IC, OP) c broadcast of prior round
            bT_prev = None    # SBUF (10, I) f32 routing logits

            for it in range(3):
                # ---------- s matmul phase ----------
                last = it == 2
                s_ps = psA.tile([BL, O, P], f32, tag="smallps")
                for s in range(S):
                    ks = slice(s * IC, (s + 1) * IC)
                    if it == 0:
                        rhs_g = wsh[:, ks, :]
                    else:
                        rhs = rhsp.tile([128, IC, OP], fp16, tag="rhs16")
                        nc.vector.tensor_tensor(
                            rhs[:], wsh[:, ks, :], cexp_prev[:], MUL
                        )
                        rhs_g = rhs
                    if s == 0:
                        # prefetch sqrt table while the PE streams
                        tsd = ring2.tile([1, 2], f32, tag=f"tsd{it}")
                        nc.scalar.activation(
                            tsd[:], tscr[:], AF.Sqrt, bias=eps_b[:1]
                        )
                    for icx in range(IC):
                        k = s * IC + icx
                        nc.tensor.matmul(
                            s_ps[:],
                            x2h[:, k, :],
                            rhs_g[:, icx, :],
                            start=(k == 0),
                            stop=(k == KT - 1),
                        )

                # ---------- squash ----------
                s_sb = ring1.tile([BL, O, P], f32, tag="s_sb")
                nc.vector.tensor_scalar_mul(
                    s_sb[:], s_ps[:], 1.0 / I if it == 0 else 1.0
                )
                s2 = ring1.tile([BL, O, P], f32, tag="s2")
                nc.vector.tensor_tensor(s2[:], s_sb[:], s_sb[:], MUL)
                sq = ring1.tile([BL, O], f32, tag="sq")
                nc.vector.tensor_reduce(sq[:], s2[:], axis=mybir.AxisListType.X, op=ADD)
                sqs = ring1.tile([BL, O], f32, tag="sqs")
                nc.scalar.activation(sqs[:], sq[:], AF.Sqrt, bias=eps_b[:BL])
                den = ring1.tile([BL, O], f32, tag="den")
                nc.vector.scalar_tensor_tensor(
                    den[:], sq[:], 1.0, sqs[:], op0=ADD, op1=MUL
                )
                rec = ring1.tile([BL, O], f32, tag="rec")
                nc.vector.reciprocal(rec[:], den[:])
                tfac = ring1.tile([BL, O], f32, tag="tfac")
                nc.vector.tensor_tensor(tfac[:], sq[:], rec[:], MUL)

                if last:
                    v_sb = ring1.tile([BL, O, P], f32, tag="v_sb")
                    nc.vector.tensor_tensor(
                        v_sb[:],
                        s_sb[:],
                        tfac[:, :, None].to_broadcast([BL, O, P]),
                        MUL,
                    )
                    nc.sync.dma_start(vout_d[:], v_sb[:])
                    continue

                nc.vector.tensor_tensor(
                    v8[:, 0:OP].rearrange("b (o p) -> b o p", o=O),
                    s_sb[:],
                    tfac[:, :, None].to_broadcast([BL, O, P]),
                    MUL,
                )

                # ---------- agree phase ----------
                # Per 384-col chunk j: two plain fp8 M2 matmuls (FWL) into a
                # 2-bank PSUM pair, one product tensor_tensor (vector, or
                # scalar-copy+gpsimd every GPS_EVERY-th chunk), and one fp8
                # DoubleRow Sel matmul accumulating agree over s. Sel is
                # emitted SEL_LAG chunks late so the PE never waits on
                # products.
                ag_ps = psG.tile([32, NB, F32MAX], f32, tag="big")

                def emit_sel(pend):
                    pa, s_idx, ib = pend
                    nc.tensor.matmul(
                        ag_ps[:, ib, 0:FB],
                        sel8[:],
                        pa[:],
                        start=(s_idx == 0),
                        stop=(s_idx == S - 1),
                        perf_mode=DR,
                    )
                    if s_idx == S - 1:
                        fsb = slice(ib * FB, (ib + 1) * FB)
                        nc.vector.tensor_copy(agP[:, fsb], ag_ps[0:O, ib, 0:FB])

                from collections import deque

                pend = deque()
                agP = ring1.tile([O, I], f32, tag="agP")
                ag_in = dram.tile([O, I], f32, tag="ag_in")
                ag_out = dram.tile([O, I], f32, tag="ag_out")
                for j in range(NJ):
                    s_idx, ib = divmod(j, NB)
                    fs = slice(j * FB, (j + 1) * FB)
                    m2 = psM.tile([128, 2, F32MAX], f32, tag="m2")
                    nc.tensor.matmul(
                        m2[:, 0, 0:FB],
                        v8[:, 0:128],
                        x3f[:, fs],
                        start=True,
                        stop=True,
                    )
                    nc.tensor.matmul(
                        m2[:, 1, 0:FB],
                        v8[:, 128:256],
                        x3f[:, fs],
                        start=True,
                        stop=True,
                    )
                    if j == 0:
                        # prefetch exp table while the PE streams the agree
                        # phase (squash's sqrt is done; copies work in any set)
                        ted = ring2.tile([1, 2], f32, tag=f"ted{it}")
                        nc.scalar.activation(
                            ted[:], tscr[:], AF.Exp, bias=zero_b[:1]
                        )
                    if len(pend) >= SEL_LAG:
                        emit_sel(pend.popleft())
                    pa = prodp.tile([128, 2, FB], fp8, tag="prod")
                    if j % GPS_EVERY == GPS_EVERY - 1 and j < NJ - 6:
                        m2c = m2cp.tile([128, 2, FB], bf16, tag="m2c")
                        nc.scalar.copy(out=m2c[:], in_=m2[:, :, 0:FB])
                        nc.gpsimd.tensor_tensor(pa[:], m2c[:], wt2[:, :, fs], MUL)
                    else:
                        nc.vector.tensor_tensor(
                            pa[:], m2[:, :, 0:FB], wt2[:, :, fs], MUL
                        )
                    pend.append((pa, s_idx, ib))
                while pend:
                    emit_sel(pend.popleft())

                # ---------- AllReduce of agree partials ----------
                nc.sync.dma_start(ag_in[:], agP[:])
                nc.gpsimd.collective_compute(
                    "AllReduce",
                    ADD,
                    replica_groups=[list(range(NCORES))],
                    ins=[ag_in.opt()],
                    outs=[ag_out.opt()],
                )
                # ---------- b update (1/B fold) + softmax, per-bank pipeline ----------
                agAR = ring2.tile([O, I], f32, tag="agAR")
                bT = ring1.tile([O, I], f32, tag=f"bT{it}")
                eT = ring1.tile([O, I], f32, tag="eT")
                esum3 = ring1.tile([O, NB], f32, tag="esum3")
                for h in range(NB):
                    cs = slice(h * FB, (h + 1) * FB)
                    nc.sync.dma_start(agAR[:, cs], ag_out[:, cs])
                    if bT_prev is None:
                        nc.vector.tensor_scalar_mul(bT[:, cs], agAR[:, cs], 1.0 / B)
                    else:
                        nc.vector.scalar_tensor_tensor(
                            bT[:, cs], agAR[:, cs], 1.0 / B, bT_prev[:, cs],
                            op0=MUL, op1=ADD,
                        )
                    nc.scalar.activation(
                        eT[:, cs],
                        bT[:, cs],
                        AF.Exp,
                        bias=zero_b[:O],
                        accum_out=esum3[:, h : h + 1],
                    )
                bT_prev = bT
                esum = ring1.tile([O, 1], f32, tag="esum")
                nc.vector.tensor_reduce(
                    esum[:], esum3[:], axis=mybir.AxisListType.X, op=ADD
                )
                erec = ring1.tile([O, 1], f32, tag="erec")
                nc.vector.reciprocal(erec[:], esum[:])

                # ---------- c_exp via broadcast matmul ----------
                # cT is produced bank-by-bank so its matmuls start before the
                # full softmax normalization completes.
                cT = ring1.tile([O, I], fp16, tag="cT")
                ce_ps = psG.tile([128, NB, F32MAX], f32, tag="big")
                cexp = cexpp.tile([128, IC, OP], fp16, tag="cexp")
                for bank in range(NB):
                    cs = slice(bank * NB * 128, (bank + 1) * NB * 128)
                    nc.vector.tensor_scalar_mul(cT[:, cs], eT[:, cs], erec[:])
                    for q in range(NB):
                        icx = bank * NB + q
                        nc.tensor.matmul(
                            ce_ps[:, bank, q * OP : (q + 1) * OP],
                            cT[:, icx * 128 : (icx + 1) * 128],
                            b2h[:],
                            start=True,
                            stop=True,
                        )
                    nc.scalar.copy(
                        out=cexp[:, bank * NB : (bank + 1) * NB, :],
                        in_=ce_ps[:, bank, 0 : NB * OP].rearrange(
                            "p (q n) -> p q n", n=OP
                        ),
                    )
                cexp_prev = cexp

    nc.compile()
    return nc


def _get_module():
    if "nc" not in _CACHE:
        _CACHE["nc"] = _build_module()
    return _CACHE["nc"]


def _prep_inputs(x, W):
    """Host-side relayouts (free: not counted in HW exec time)."""
    f8 = ml_dtypes.float8_e4m3
    f8w = ml_dtypes.float8_e3m4
    f16 = np.float16
    x = np.ascontiguousarray(np.asarray(x, np.float32))
    W = np.ascontiguousarray(np.asarray(W, np.float32))

    x2 = x.transpose(2, 1, 0).reshape(IS, B)           # [(s,i), b]
    x2t = x2.reshape(KT, 128, B).transpose(1, 0, 2)    # [p, k, b]
    x3 = x.transpose(0, 2, 1).reshape(B, IS)           # [b, (s,i)]
    ws = W.transpose(3, 0, 1, 2).reshape(IS, OP)       # [(s,i), (o,p)]
    wst = ws.reshape(KT, 128, OP).transpose(1, 0, 2)   # [p, k, (o,p)]
    wt = W.transpose(1, 2, 3, 0).reshape(OP, IS)       # [(o,p), (s,i)]

    wt2 = np.zeros((128, 2, IS), np.float32)
    wt2[:, 0, :] = wt[0:128]
    wt2[0:32, 1, :] = wt[128:160]

    sel8 = np.zeros((128, 2, 32), np.float32)
    for o in range(O):
        sel8[o * P : (o + 1) * P, 0, o] = 1.0
    for r in range(32):
        sel8[r, 1, (128 + r) // P] = 1.0

    b2 = np.zeros((O, OP), np.float32)
    for o in range(O):
        b2[o, o * P : (o + 1) * P] = 1.0

    shared = {
        "wsh": np.ascontiguousarray(wst).astype(f16),
        "wt2": wt2.astype(f8w),
        "sel8": sel8.astype(f8),
        "b2h": b2.astype(f16),
    }
    in_maps = []
    for c in range(NCORES):
        bs = slice(c * BL, (c + 1) * BL)
        m = dict(shared)
        m["x2h"] = np.ascontiguousarray(x2t[:, :, bs]).astype(f16)
        m["x3f"] = np.ascontiguousarray(x3[bs]).astype(ml_dtypes.bfloat16)
        in_maps.append(m)
    return in_maps


def run(x, W, trace=False, tmpdir=None):
    import jax
    from concourse import bass_utils

    try:
        jax.config.update("jax_compilation_cache_dir", "/tmp/jax_neff_cache")
        jax.config.update("jax_persistent_cache_min_compile_time_secs", 1.0)
    except Exception:
        pass

    nc = _get_module()
    in_maps = _prep_inputs(x, W)
    res = bass_utils.run_bass_kernel_spmd(
        nc, in_maps, core_ids=list(range(NCORES)), trace=trace, tmpdir=tmpdir
    )
    v = np.concatenate([res.results[c]["vout"] for c in range(NCORES)], axis=0)
    return v.reshape(B, O, P).astype(np.float32), res


def kernel(x, W):
    v, _ = run(x, W)
    return v


# revision 11
# speedup vs baseline: 1.0231x; 1.0231x over previous
"""DigitCaps dynamic-routing kernel for Trainium2 (8 NeuronCores, Bass/Tile).

Strategy (pure batch data-parallelism, 64 batch rows per core):
  u_hat (B,1152,10,16) is NEVER materialized. Per routing iteration:

    s[b,(o,p)]   = sum_k x2[k,b] * (c ⊙ Ws)[k,(o,p)]      (72 fp16 K-tile matmuls)
    M2[(o,p),k]  = sum_b v[b,(o,p)] * x3[b,k]             (fp8 matmuls, FWL path;
                   the 160 (o,p) rows live as 128 + 32-padded-to-128 in two
                   PSUM banks)
    agree[o,i]   = Sel^T @ (W ⊙ M2)  with (p,s)-reduction on the PE
                   (one tensor_tensor product + one fp8 DoubleRow Sel matmul
                   per 384-col chunk, accumulated over s in PSUM)

  The only cross-core op is an AllReduce of the (10,1152) agree partials per
  routing iteration (2 total), preceded by a same-sized warmup AllReduce that
  absorbs first-collective setup.

Precision: s-matmuls in fp16 for all 3 iterations; the agree path (v, x3,
products) in fp8e4m3, W-product operand in fp8e3m4 — this error only perturbs
the routing logits b_ij and stays well inside the 2e-2 budget.

Scalar-engine activation tables: exp and sqrt live in different table sets, so
each switch costs ~1.3us. Dummy activations prefetch the upcoming set while
the scalar engine is idle, keeping table loads off the serial chains.
"""
import sys

sys.path.insert(0, "/opt/trn_rl_repo")

import numpy as np
import ml_dtypes

# ---- problem constants (hardcoded per harness contract) ----
B, I, S, O, P = 512, 1152, 8, 10, 16
IS = I * S            # 9216  contraction size, k = s*I + i
OP = O * P            # 160
NCORES = 8
BL = B // NCORES      # 64 batch rows per core
KT = IS // 128        # 72 K-tiles
IC = I // 128         # 9 i-chunks per s-group
FB = 384              # free-chunk width of the agree pipeline (I = 3*FB)
NJ = IS // FB         # 24 chunks, j = s*3 + ib
NB = 3                # i-blocks for agree PSUM accumulation
F32MAX = 512          # PSUM bank width in f32 elements

GPS_EVERY = 3         # every GPS_EVERY-th product chunk goes scalar-copy+gpsimd
SEL_LAG = 5           # chunks between product issue and its Sel matmul

_CACHE = {}


def _build_module():
    import concourse.bass as bass
    import concourse.mybir as mybir
    import concourse.tile as tile
    from concourse import bacc

    f32 = mybir.dt.float32
    bf16 = mybir.dt.bfloat16
    fp16 = mybir.dt.float16
    fp8 = mybir.dt.float8e4
    fp8w = mybir.dt.float8e3
    MUL = mybir.AluOpType.mult
    ADD = mybir.AluOpType.add
    DR = mybir.MatmulPerfMode.DoubleRow
    AF = mybir.ActivationFunctionType

    nc = bacc.Bacc(
        "TRN2",
        target_bir_lowering=False,
        debug=False,
        num_devices=NCORES,
    )

    # ---- I/O ----
    x2h_d = nc.dram_tensor("x2h", [128, KT, BL], fp16, kind="ExternalInput")
    wsh_d = nc.dram_tensor("wsh", [128, KT, OP], fp16, kind="ExternalInput")
    x3f_d = nc.dram_tensor("x3f", [BL, IS], fp8, kind="ExternalInput")
    wt2_d = nc.dram_tensor("wt2", [128, 2, IS], fp8w, kind="ExternalInput")
    sel8_d = nc.dram_tensor("sel8", [128, 2, 32], fp8, kind="ExternalInput")
    b2h_d = nc.dram_tensor("b2h", [O, OP], fp16, kind="ExternalInput")
    vout_d = nc.dram_tensor("vout", [BL, OP], f32, kind="ExternalOutput")

    with tile.TileContext(nc) as tc:
        with (
            tc.tile_pool(name="const", bufs=1) as const,
            tc.tile_pool(name="rhsbig", bufs=3) as rhsp,
            tc.tile_pool(name="prod", bufs=8) as prodp,
            tc.tile_pool(name="m2c", bufs=2) as m2cp,
            tc.tile_pool(name="cexp", bufs=2) as cexpp,
            tc.tile_pool(name="ring1", bufs=1) as ring1,
            tc.tile_pool(name="ring2", bufs=2) as ring2,
            tc.tile_pool(name="psA", bufs=1, space="PSUM") as psA,
            tc.tile_pool(name="psM", bufs=2, space="PSUM") as psM,
            tc.tile_pool(name="psG", bufs=1, space="PSUM") as psG,
            tc.tile_pool(name="dram", bufs=2, space="DRAM") as dram,
        ):
            # ---------- persistent tiles ----------
            x2h = const.tile([128, KT, BL], fp16)
            wsh = const.tile([128, KT, OP], fp16)
            x3f = const.tile([BL, IS], fp8)
            wt2 = const.tile([128, 2, IS], fp8w)
            sel8 = const.tile([128, 2, 32], fp8)
            b2h = const.tile([O, OP], fp16)
            v8 = const.tile([BL, 2 * 128], fp8)  # squash out + zero pad

            # Sized warmup AllReduce input staged FIRST on the sync queue so
            # the warmup collective runs right after the entry barrier,
            # leaving the cc stream free for the real agree AllReduce.
            warm_sb = const.tile([O, I], f32)
            nc.vector.memset(warm_sb[:], 0.0)
            warm_in = dram.tile([O, I], f32, tag="warm_in")
            warm_out = dram.tile([O, I], f32, tag="warm_out")
            nc.sync.dma_start(warm_in[:], warm_sb[:])
            nc.gpsimd.collective_compute(
                "AllReduce",
                ADD,
                replica_groups=[list(range(NCORES))],
                ins=[warm_in.opt()],
                outs=[warm_out.opt()],
            )

            # ---------- load inputs (3 parallel queues) ----------
            # scalar/gpsimd: wsh alternating s-groups (the iter0 critical
            # stream); sync: x2h then agree-phase operands.
            nc.gpsimd.dma_start(sel8[:], sel8_d[:])
            nc.gpsimd.dma_start(b2h[:], b2h_d[:])
            for q0 in range(0, 3):
                ks = slice(q0 * 3, (q0 + 1) * 3)
                nc.scalar.dma_start(wsh[:, ks, :], wsh_d[:, ks, :])
            for s in range(1, S):
                ks = slice(s * IC, (s + 1) * IC)
                eng = nc.scalar if s % 2 == 0 else nc.gpsimd
                eng.dma_start(wsh[:, ks, :], wsh_d[:, ks, :])
            for s in range(S):
                ks = slice(s * IC, (s + 1) * IC)
                nc.sync.dma_start(x2h[:, ks, :], x2h_d[:, ks, :])
            nc.sync.dma_start(x3f[:], x3f_d[:])
            JCH = 3 * FB
            for c0 in range(0, IS, JCH):
                cs = slice(c0, c0 + JCH)
                nc.sync.dma_start(wt2[:, :, cs], wt2_d[:, :, cs])

            # zero pad region of v8 (persists across iterations)
            nc.vector.memset(v8[:, OP:], 0.0)

            # bias APs for activation (float biases need pre-registered consts)
            zero_b = const.tile([128, 1], f32)
            eps_b = const.tile([128, 1], f32)
            nc.vector.memset(zero_b[:], 0.0)
            nc.vector.memset(eps_b[:], 1e-8)
            # scratch for activation-table prefetch dummies
            tscr = const.tile([1, 2], f32)
            nc.vector.memset(tscr[:], 1.0)

            cexp_prev = None  # fp16 (128, IC, OP) c broadcast of prior round
            bT_prev = None    # SBUF (10, I) f32 routing logits

            for it in range(3):
                # ---------- s matmul phase ----------
                last = it == 2
                s_ps = psA.tile([BL, O, P], f32, tag="smallps")
                for s in range(S):
                    ks = slice(s * IC, (s + 1) * IC)
                    if it == 0:
                        rhs_g = wsh[:, ks, :]
                    else:
                        rhs = rhsp.tile([128, IC, OP], fp16, tag="rhs16")
                        nc.vector.tensor_tensor(
                            rhs[:], wsh[:, ks, :], cexp_prev[:], MUL
                        )
                        rhs_g = rhs
                    if s == 0:
                        # prefetch sqrt table while the PE streams
                        tsd = ring2.tile([1, 2], f32, tag=f"tsd{it}")
                        nc.scalar.activation(
                            tsd[:], tscr[:], AF.Sqrt, bias=eps_b[:1]
                        )
                    for icx in range(IC):
                        k = s * IC + icx
                        nc.tensor.matmul(
                            s_ps[:],
                            x2h[:, k, :],
                            rhs_g[:, icx, :],
                            start=(k == 0),
                            stop=(k == KT - 1),
                        )

                # ---------- squash ----------
                s_sb = ring1.tile([BL, O, P], f32, tag="s_sb")
                nc.vector.tensor_scalar_mul(
                    s_sb[:], s_ps[:], 1.0 / I if it == 0 else 1.0
                )
                s2 = ring1.tile([BL, O, P], f32, tag="s2")
                nc.vector.tensor_tensor(s2[:], s_sb[:], s_sb[:], MUL)
                sq = ring1.tile([BL, O], f32, tag="sq")
                nc.vector.tensor_reduce(sq[:], s2[:], axis=mybir.AxisListType.X, op=ADD)
                sqs = ring1.tile([BL, O], f32, tag="sqs")
                nc.scalar.activation(sqs[:], sq[:], AF.Sqrt, bias=eps_b[:BL])
                den = ring1.tile([BL, O], f32, tag="den")
                nc.vector.scalar_tensor_tensor(
                    den[:], sq[:], 1.0, sqs[:], op0=ADD, op1=MUL
                )
                rec = ring1.tile([BL, O], f32, tag="rec")
                nc.vector.reciprocal(rec[:], den[:])
                tfac = ring1.tile([BL, O], f32, tag="tfac")
                nc.vector.tensor_tensor(tfac[:], sq[:], rec[:], MUL)

                if last:
                    v_sb = ring1.tile([BL, O, P], f32, tag="v_sb")
                    nc.vector.tensor_tensor(
                        v_sb[:],
                        s_sb[:],
                        tfac[:, :, None].to_broadcast([BL, O, P]),
                        MUL,
                    )
                    nc.sync.dma_start(vout_d[:], v_sb[:])
                    continue

                nc.vector.tensor_tensor(
                    v8[:, 0:OP].rearrange("b (o p) -> b o p", o=O),
                    s_sb[:],
                    tfac[:, :, None].to_broadcast([BL, O, P]),
                    MUL,
                )

                # ---------- agree phase ----------
                # Per 384-col chunk j: two plain fp8 M2 matmuls (FWL) into a
                # 2-bank PSUM pair, one product tensor_tensor (vector, or
                # scalar-copy+gpsimd every GPS_EVERY-th chunk), and one fp8
                # DoubleRow Sel matmul accumulating agree over s. Sel is
                # emitted SEL_LAG chunks late so the PE never waits on
                # products.
                ag_ps = psG.tile([32, NB, F32MAX], f32, tag="big")

                def emit_sel(pend):
                    pa, s_idx, ib = pend
                    nc.tensor.matmul(
                        ag_ps[:, ib, 0:FB],
                        sel8[:],
                        pa[:],
                        start=(s_idx == 0),
                        stop=(s_idx == S - 1),
                        perf_mode=DR,
                    )
                    if s_idx == S - 1:
                        fsb = slice(ib * FB, (ib + 1) * FB)
                        nc.vector.tensor_copy(agP[:, fsb], ag_ps[0:O, ib, 0:FB])

                from collections import deque

                pend = deque()
                agP = ring1.tile([O, I], f32, tag="agP")
                ag_in = dram.tile([O, I], f32, tag="ag_in")
                ag_out = dram.tile([O, I], f32, tag="ag_out")
                for j in range(NJ):
                    s_idx, ib = divmod(j, NB)
                    fs = slice(j * FB, (j + 1) * FB)
                    m2 = psM.tile([128, 2, F32MAX], f32, tag="m2")
                    nc.tensor.matmul(
                        m2[:, 0, 0:FB],
                        v8[:, 0:128],
                        x3f[:, fs],
                        start=True,
                        stop=True,
                    )
                    nc.tensor.matmul(
                        m2[:, 1, 0:FB],
                        v8[:, 128:256],
                        x3f[:, fs],
                        start=True,
                        stop=True,
                    )
                    if j == 0:
                        # prefetch exp table while the PE streams the agree
                        # phase (squash's sqrt is done; copies work in any set)
                        ted = ring2.tile([1, 2], f32, tag=f"ted{it}")
                        nc.scalar.activation(
                            ted[:], tscr[:], AF.Exp, bias=zero_b[:1]
                        )
                    if len(pend) >= SEL_LAG:
                        emit_sel(pend.popleft())
                    pa = prodp.tile([128, 2, FB], fp8, tag="prod")
                    if j % GPS_EVERY == GPS_EVERY - 1 and j < NJ - 6:
                        m2c = m2cp.tile([128, 2, FB], bf16, tag="m2c")
                        nc.scalar.copy(out=m2c[:], in_=m2[:, :, 0:FB])
                        nc.gpsimd.tensor_tensor(pa[:], m2c[:], wt2[:, :, fs], MUL)
                    else:
                        nc.vector.tensor_tensor(
                            pa[:], m2[:, :, 0:FB], wt2[:, :, fs], MUL
                        )
                    pend.append((pa, s_idx, ib))
                while pend:
                    emit_sel(pend.popleft())

                # ---------- AllReduce of agree partials ----------
                nc.sync.dma_start(ag_in[:], agP[:])
                nc.gpsimd.collective_compute(
                    "AllReduce",
                    ADD,
                    replica_groups=[list(range(NCORES))],
                    ins=[ag_in.opt()],
                    outs=[ag_out.opt()],
                )
                agAR = ring2.tile([O, I], f32, tag="agAR")
                nc.sync.dma_start(agAR[:], ag_out[:])

                # ---------- b update (with 1/B fold) + softmax over i ----------
                bT = ring1.tile([O, I], f32, tag=f"bT{it}")
                if bT_prev is None:
                    nc.vector.tensor_scalar_mul(bT[:], agAR[:], 1.0 / B)
                else:
                    nc.vector.scalar_tensor_tensor(
                        bT[:], agAR[:], 1.0 / B, bT_prev[:], op0=MUL, op1=ADD
                    )
                bT_prev = bT

                eT = ring1.tile([O, I], f32, tag="eT")
                esum = ring1.tile([O, 1], f32, tag="esum")
                nc.scalar.activation(
                    eT[:],
                    bT[:],
                    AF.Exp,
                    bias=zero_b[:O],
                    accum_out=esum[:],
                )
                erec = ring1.tile([O, 1], f32, tag="erec")
                nc.vector.reciprocal(erec[:], esum[:])

                # ---------- c_exp via broadcast matmul ----------
                # cT is produced bank-by-bank so its matmuls start before the
                # full softmax normalization completes.
                cT = ring1.tile([O, I], fp16, tag="cT")
                ce_ps = psG.tile([128, NB, F32MAX], f32, tag="big")
                cexp = cexpp.tile([128, IC, OP], fp16, tag="cexp")
                for bank in range(NB):
                    cs = slice(bank * NB * 128, (bank + 1) * NB * 128)
                    nc.vector.tensor_scalar_mul(cT[:, cs], eT[:, cs], erec[:])
                    for q in range(NB):
                        icx = bank * NB + q
                        nc.tensor.matmul(
                            ce_ps[:, bank, q * OP : (q + 1) * OP],
                            cT[:, icx * 128 : (icx + 1) * 128],
                            b2h[:],
                            start=True,
                            stop=True,
                        )
                    nc.scalar.copy(
                        out=cexp[:, bank * NB : (bank + 1) * NB, :],
                        in_=ce_ps[:, bank, 0 : NB * OP].rearrange(
                            "p (q n) -> p q n", n=OP
                        ),
                    )
                cexp_prev = cexp

    nc.compile()
    return nc


def _get_module():
    if "nc" not in _CACHE:
        _CACHE["nc"] = _build_module()
    return _CACHE["nc"]


def _prep_inputs(x, W):
    """Host-side relayouts (free: not counted in HW exec time)."""
    f8 = ml_dtypes.float8_e4m3
    f8w = ml_dtypes.float8_e3m4
    f16 = np.float16
    x = np.ascontiguousarray(np.asarray(x, np.float32))
    W = np.ascontiguousarray(np.asarray(W, np.float32))

    x2 = x.transpose(2, 1, 0).reshape(IS, B)           # [(s,i), b]
    x2t = x2.reshape(KT, 128, B).transpose(1, 0, 2)    # [p, k, b]
    x3 = x.transpose(0, 2, 1).reshape(B, IS)           # [b, (s,i)]
    ws = W.transpose(3, 0, 1, 2).reshape(IS, OP)       # [(s,i), (o,p)]
    wst = ws.reshape(KT, 128, OP).transpose(1, 0, 2)   # [p, k, (o,p)]
    wt = W.transpose(1, 2, 3, 0).reshape(OP, IS)       # [(o,p), (s,i)]

    wt2 = np.zeros((128, 2, IS), np.float32)
    wt2[:, 0, :] = wt[0:128]
    wt2[0:32, 1, :] = wt[128:160]

    sel8 = np.zeros((128, 2, 32), np.float32)
    for o in range(O):
        sel8[o * P : (o + 1) * P, 0, o] = 1.0
    for r in range(32):
        sel8[r, 1, (128 + r) // P] = 1.0

    b2 = np.zeros((O, OP), np.float32)
    for o in range(O):
        b2[o, o * P : (o + 1) * P] = 1.0

    shared = {
        "wsh": np.ascontiguousarray(wst).astype(f16),
        "wt2": wt2.astype(f8w),
        "sel8": sel8.astype(f8),
        "b2h": b2.astype(f16),
    }
    in_maps = []
    for c in range(NCORES):
        bs = slice(c * BL, (c + 1) * BL)
        m = dict(shared)
        m["x2h"] = np.ascontiguousarray(x2t[:, :, bs]).astype(f16)
        m["x3f"] = np.ascontiguousarray(x3[bs]).astype(f8)
        in_maps.append(m)
    return in_maps


def run(x, W, trace=False, tmpdir=None):
    import jax
    from concourse import bass_utils

    try:
        jax.config.update("jax_compilation_cache_dir", "/tmp/jax_neff_cache")
        jax.config.update("jax_persistent_cache_min_compile_time_secs", 1.0)
    except Exception:
        pass

    nc = _get_module()
    in_maps = _prep_inputs(x, W)
    res = bass_utils.run_bass_kernel_spmd(
        nc, in_maps, core_ids=list(range(NCORES)), trace=trace, tmpdir=tmpdir
    )
    v = np.concatenate([res.results[c]["vout"] for c in range(NCORES)], axis=0)
    return v.reshape(B, O, P).astype(np.float32), res


def kernel(x, W):
    v, _ = run(x, W)
    return v
